# revision 89
# baseline (speedup 1.0000x reference)
"""Trainium2 Bass kernel: 3x3 VALID conv (NHWC) with weight thresholding + bias.

Full-input contract: kernel(x, weight, bias) -> out
  x:      (32, 56, 56, 256) fp32 NHWC
  weight: (256, 256, 3, 3)  fp32 OIHW, |w| < 0.01 -> 0
  bias:   (256,)            fp32
  out:    (32, 54, 54, 256) fp32 NHWC
Sharding: data-parallel over batch, 4 images per core on 8 cores.

Device algorithm: 1D Winograd F(3,3) along the width axis (nodes 0,1,-1,2,inf
-> 5 multiplies per 3 outputs vs 9 dense, a 1.8x PE reduction), dense shifted
PSUM accumulation along height, implicit GEMM over channels. Per width-tile t
(outputs 3t..3t+2), with d_l = x[3t+l]:
  V3 = d3-d1, e = d0-d2, V0 = 2e+V3, t1 = d3-2d1, V1 = t1-d2,
  t2 = d3-3d2, V2 = 2d1+t2, f = d4-d2, V4 = f-2*V3          (input transform)
  M_j[co, oh, t] = sum_{kh, ci} U_j,kh[ci, co] V_j[ci, oh+kh, t]   (PE)
  y0 = M0+M1+M2+M3+b, y1 = M1-M2+2M3+b, y2 = M1+M2+4M3+M4+b  (output transform)
U = (G w) along kw with G = [[1/2,0,0],[-1/2,-1/2,-1/2],[-1/6,1/6,-1/6],
[1/6,1/3,2/3],[0,0,1]] (host-precomputed, bf16).

Engine split: the 5 first-level input-transform ops run on Pool (gpsimd,
plain tensor_tensor only -- no TensorScalarPtr opcode there), the 4
second-level scaled ops on DVE; image 0's level-1 stays on DVE because
Pool's latency would gate startup. For the output transform, ACT copies all
five PSUM banks to bf16 SBUF (Identity activation; bias folded into the M1
copy) -- DVE may read at most ONE PSUM operand per op and Pool none at all,
so this frees banks early and turns the 7 DVE combines into cheap SBUF-only
bf16 ops. Output DMAs ride the SP queue: on ACT's in-order queue their
waits would stall the next unit's PSUM-freeing copies.

Everything device-side is bf16; PSUM accumulates fp32.
Host converts x to bf16, de-interleaves width mod 3 (so all transform reads
are row-contiguous), and converts the bf16 output back to fp32. A stream of
tiny dummy matmuls during the startup DMA window pre-warms the PE clock (HAM)
so the real matmul stream starts at full rate.
"""

import numpy as np
import ml_dtypes
from contextlib import ExitStack

import concourse.bass as bass
import concourse.bacc as bacc
import concourse.tile as tile
import concourse.mybir as mybir
from concourse.bass_utils import run_bass_kernel_spmd

N_CORES = 8
IMGS_PER_CORE = 4
H, W, C = 56, 56, 256
OH, OW, CO = 54, 54, 256
P = 128
NW = 19              # padded columns per width residue class (mod 3)
NT = OW // 3         # 18 winograd tiles per row
RCOLS = H * NW       # 1064 cols per (ci, residue)
ROWS_PER_BLK = 27    # 27 output rows * 18 tiles = 486 <= 512 (one PSUM bank)
N_BLKS = OH // ROWS_PER_BLK  # 2
BLK = ROWS_PER_BLK * NT      # 486
SPARSE_TH = 0.01

XCOLS_IMG = 2 * 3 * RCOLS     # ci(2) x r(3) x 1064 = 6384
VCOLS_CI = 5 * H * NT         # pos(5) x h(56) x t(18) = 5040
YCOLS_IMG = N_BLKS * 3 * BLK  # blk(2) x i(3) x 486 = 2916

TRACE = False
LAST = None
SIM_NS = None

_NC_CACHE = None
_last_in_maps = None

bf16 = mybir.dt.bfloat16
f32 = mybir.dt.float32


def _build_module():
    nc = bacc.Bacc(
        "TRN2",
        target_bir_lowering=False,
        debug=False,
        enable_asserts=False,
        num_devices=N_CORES,
    )
    xt = nc.dram_tensor("xt", [P, IMGS_PER_CORE * XCOLS_IMG], bf16, kind="ExternalInput").ap()
    up = nc.dram_tensor("up", [P, 60 * P], bf16, kind="ExternalInput").ap()
    b2 = nc.dram_tensor("b2", [P, 2], f32, kind="ExternalInput").ap()
    yt = nc.dram_tensor("yt", [CO, IMGS_PER_CORE * YCOLS_IMG], bf16, kind="ExternalOutput").ap()

    add = mybir.AluOpType.add
    sub = mybir.AluOpType.subtract
    mult = mybir.AluOpType.mult

    with tile.TileContext(nc) as tc, ExitStack() as ctx:
        upool = ctx.enter_context(tc.tile_pool(name="u", bufs=1))
        bpool = ctx.enter_context(tc.tile_pool(name="b", bufs=1))
        xpool = ctx.enter_context(tc.tile_pool(name="x", bufs=2))
        vpool = ctx.enter_context(tc.tile_pool(name="v", bufs=2))
        itpool = ctx.enter_context(tc.tile_pool(name="it", bufs=8))
        tpool = ctx.enter_context(tc.tile_pool(name="t", bufs=8))
        opool = ctx.enter_context(tc.tile_pool(name="o", bufs=6))
        pspool = ctx.enter_context(tc.tile_pool(name="ps", bufs=8, space="PSUM"))

        u_sb = upool.tile([P, 60 * P], bf16)
        b_sb = bpool.tile([P, 2], f32)

        # PE ramp warmup (HAM): fill the startup DMA wait with tiny matmuls
        NWARM = 80
        w_warm = upool.tile([P, 64], bf16)
        nc.gpsimd.memset(w_warm[:], 0.0)
        ps_warm = pspool.tile([P, BLK], f32, tag="ps", name="ps_warm")
        for i in range(NWARM):
            nc.tensor.matmul(ps_warm[:64, :64], w_warm[:], w_warm[:],
                             start=(i == 0), stop=(i == NWARM - 1))

        # weight block index within u_sb: co-major so co=0 weights DMA first
        def tblk(co, pos, kh, ci):
            return ((co * 5 + pos) * 3 + kh) * 2 + ci

        def load_u(co, pos0, pos1):
            c0 = tblk(co, pos0, 0, 0) * P
            c1 = (tblk(co, pos1, 2, 1) + 1) * P
            nc.scalar.dma_start(u_sb[:, c0:c1], up[:, c0:c1])

        # x chunking: rows [0,30) cover blk0 (needs rows 0..28), [30,56) rest
        RSPLIT = 30

        def load_x_rows(x_tile, img, h0, h1):
            """One DMA: rows [h0,h1) of all 6 (ci, r) blocks (3D strided AP)."""
            src = xt[:, img * XCOLS_IMG:(img + 1) * XCOLS_IMG].rearrange(
                "p (b r) -> p b r", b=6)[:, :, h0 * NW:h1 * NW]
            dst = x_tile[:].rearrange("p (b r) -> p b r", b=6)[:, :, h0 * NW:h1 * NW]
            nc.sync.dma_start(dst, src)

        x0 = xpool.tile([P, XCOLS_IMG], bf16, tag="x", name="x_0")
        load_x_rows(x0, 0, 0, RSPLIT)
        load_u(0, 3, 3)                # matmul pos order is 3,1,2,0,4
        nc.sync.dma_start(b_sb[:], b2)
        load_x_rows(x0, 0, RSPLIT, H)
        load_u(0, 1, 2)
        load_u(0, 0, 0)
        load_u(0, 4, 4)
        load_u(1, 0, 4)

        for img in range(IMGS_PER_CORE):
            if img == 0:
                xc = x0
            else:
                xc = xpool.tile([P, XCOLS_IMG], bf16, tag="x", name=f"x_{img}")
                load_x_rows(xc, img, 0, H)

            v = vpool.tile([P, 2 * VCOLS_CI], bf16, tag="v", name=f"v_{img}")

            def vslice(ci, pos, r0, r1):
                a = ci * VCOLS_CI + pos * (H * NT)
                return v[:, a + r0 * NT:a + r1 * NT]

            def xr(ci, r):
                a = (ci * 3 + r) * RCOLS
                return xc[:, a:a + RCOLS].rearrange("p (h w) -> p h w", w=NW)

            # input transform, per row-chunk per ci, ordered by first consumer
            # (matmul pos order is 3,1,2,0,4). Level-1 (reads x) runs on Pool
            # except for image 0, where DVE is faster and Pool's latency would
            # stall startup; level-2 always on DVE.
            lvl1 = nc.vector if img == 0 else nc.gpsimd
            for (r0, r1) in ((0, RSPLIT), (RSPLIT, H)):
                nr = r1 - r0
                dd_ = {}
                it_ = {}
                for ci in range(2):
                    dd_[ci] = dict(
                        d0=xr(ci, 0)[:, r0:r1, 0:NT],
                        d1=xr(ci, 1)[:, r0:r1, 0:NT],
                        d2=xr(ci, 2)[:, r0:r1, 0:NT],
                        d3=xr(ci, 0)[:, r0:r1, 1:NT + 1],
                        d4=xr(ci, 1)[:, r0:r1, 1:NT + 1],
                    )
                    sfx = f"{img}_{r0}_{ci}"
                    it_[ci] = {
                        k: itpool.tile([P, nr * NT], bf16, tag="it",
                                       name=f"{k}_{sfx}")
                        for k in ("e", "q", "r", "f")
                    }

                def w3(ap):
                    return ap.rearrange("p (h w) -> p h w", w=NT)

                # level-1 is plain tensor_tensor only (Pool has no
                # TensorScalarPtr opcode); scales live in DVE's stt ops
                for ci in range(2):   # V3 = d3 - d1 (feeds pos3 matmuls)
                    lvl1.tensor_tensor(w3(vslice(ci, 3, r0, r1)),
                                       dd_[ci]["d3"], dd_[ci]["d1"], sub)
                for ci in range(2):   # q = d3 - d2
                    lvl1.tensor_tensor(w3(it_[ci]["q"][:]),
                                       dd_[ci]["d3"], dd_[ci]["d2"], sub)
                for ci in range(2):   # V1 = q - 2 d1
                    nc.vector.scalar_tensor_tensor(w3(vslice(ci, 1, r0, r1)),
                                                   dd_[ci]["d1"], -2.0,
                                                   w3(it_[ci]["q"][:]), mult, add)
                for ci in range(2):   # r = d1 - d2
                    lvl1.tensor_tensor(w3(it_[ci]["r"][:]),
                                       dd_[ci]["d1"], dd_[ci]["d2"], sub)
                for ci in range(2):   # V2 = 2 r + q
                    nc.vector.scalar_tensor_tensor(it_[ci]["r"][:] if False else w3(vslice(ci, 2, r0, r1)),
                                                   w3(it_[ci]["r"][:]), 2.0,
                                                   w3(it_[ci]["q"][:]), mult, add)
                for ci in range(2):   # e = d0 - d2
                    lvl1.tensor_tensor(w3(it_[ci]["e"][:]),
                                       dd_[ci]["d0"], dd_[ci]["d2"], sub)
                for ci in range(2):   # V0 = 2 e + V3
                    nc.vector.scalar_tensor_tensor(vslice(ci, 0, r0, r1),
                                                   it_[ci]["e"][:], 2.0,
                                                   vslice(ci, 3, r0, r1), mult, add)
                for ci in range(2):   # f = d4 - d2
                    lvl1.tensor_tensor(w3(it_[ci]["f"][:]),
                                       dd_[ci]["d4"], dd_[ci]["d2"], sub)
                for ci in range(2):   # V4 = f - 2 V3
                    nc.vector.scalar_tensor_tensor(vslice(ci, 4, r0, r1),
                                                   vslice(ci, 3, r0, r1), -2.0,
                                                   it_[ci]["f"][:], mult, add)

            def unit(blk, co, r_off, nrows, tag2=""):
                """Matmuls + output transform for output rows
                [blk*27+r_off, +nrows) of co-chunk `co`."""
                oh0 = blk * ROWS_PER_BLK + r_off
                n = nrows * NT
                sfx = f"{img}_{blk}_{co}{tag2}"
                ps = [
                    pspool.tile([P, n], f32, tag="ps", name=f"ps_{sfx}_{pos}")
                    for pos in range(5)
                ]
                for pos in (3, 1, 2, 0, 4):   # matches V readiness order
                    mm = 0
                    for kh in range(3):
                        for ci in range(2):
                            t = tblk(co, pos, kh, ci)
                            rhs = vslice(ci, pos, oh0 + kh, oh0 + kh + nrows)
                            nc.tensor.matmul(
                                ps[pos][:],
                                u_sb[:, t * P:(t + 1) * P],
                                rhs,
                                start=(mm == 0),
                                stop=(mm == 5),
                            )
                            mm += 1
                # output transform + bias. ACT (otherwise idle) copies
                # every PSUM bank to bf16 SBUF -- freeing banks early --
                # and folds the bias into the m1 copy; the 7 DVE combines
                # are then cheap SBUF-only bf16 ops.
                bias = b_sb[:, co:co + 1]
                c = {
                    k: tpool.tile([P, n], bf16, tag="t", name=f"{k}_{sfx}")
                    for k in ("s1", "c0", "c2", "c3", "c4", "s", "dd", "w0", "w2")
                }
                Id = mybir.ActivationFunctionType.Identity
                yo = opool.tile([P, 3 * n], bf16, tag="yo", name=f"y_{sfx}")
                nc.scalar.activation(c["c3"][:], ps[3][:], Id, bias=0.0, scale=1.0)
                nc.scalar.activation(c["s1"][:], ps[1][:], Id, bias=bias, scale=1.0)  # m1+b
                nc.scalar.activation(c["c2"][:], ps[2][:], Id, bias=0.0, scale=1.0)
                nc.scalar.activation(c["c0"][:], ps[0][:], Id, bias=0.0, scale=1.0)
                nc.scalar.activation(c["c4"][:], ps[4][:], Id, bias=0.0, scale=1.0)
                nc.vector.tensor_tensor(c["s"][:], c["c2"][:], c["s1"][:], add)   # s = m2+m1+b
                nc.vector.scalar_tensor_tensor(c["dd"][:], c["c2"][:], -1.0, c["s1"][:], mult, add)  # dd = m1-m2+b
                nc.vector.tensor_tensor(c["w0"][:], c["c0"][:], c["s"][:], add)   # w0 = m0+s
                nc.vector.tensor_tensor(yo[:, :n], c["c3"][:], c["w0"][:], add)             # y0
                nc.vector.scalar_tensor_tensor(yo[:, n:2 * n], c["c3"][:], 2.0, c["dd"][:], mult, add)  # y1
                nc.vector.scalar_tensor_tensor(c["w2"][:], c["c3"][:], 4.0, c["s"][:], mult, add)  # w2 = 4m3+s
                nc.vector.tensor_tensor(yo[:, 2 * n:], c["c4"][:], c["w2"][:], add)         # y2
                # y DMAs ride the SP queue: on the ACT queue their in-order
                # dispatch (waiting on DVE's y2) would stall the next
                # unit's PSUM-freeing ACT copies.
                col0 = img * YCOLS_IMG + blk * 3 * BLK + r_off * NT
                if nrows == ROWS_PER_BLK:
                    nc.sync.dma_start(yt[co * P:(co + 1) * P, col0:col0 + 2 * BLK],
                                      yo[:, :2 * BLK])
                    nc.sync.dma_start(yt[co * P:(co + 1) * P, col0 + 2 * BLK:col0 + 3 * BLK],
                                      yo[:, 2 * BLK:])
                else:
                    # one 3D strided DMA covers all three output phases
                    q0 = img * 2 * 3 + blk * 3
                    off = r_off * NT
                    dst = yt[co * P:(co + 1) * P, :].rearrange(
                        "p (q r) -> p q r", r=BLK)[:, q0:q0 + 3, off:off + n]
                    nc.sync.dma_start(dst, yo[:].rearrange("p (i r) -> p i r", i=3))

            for blk in range(N_BLKS):
                for co in range(2):
                    if (img == IMGS_PER_CORE - 1 and blk == N_BLKS - 1
                            and co == 1):
                        # final unit row-split: shrinks the post-last-matmul
                        # drain chain (ACT copy + y2 + DMA scale with cols)
                        unit(blk, co, 0, 20, "a")
                        unit(blk, co, 20, 7, "b")
                    else:
                        unit(blk, co, 0, ROWS_PER_BLK)
    nc.compile()
    return nc


def _marshal(x, weight, bias):
    """Host-side sharding + layout. Returns per-core input maps."""
    x = np.ascontiguousarray(np.asarray(x, dtype=np.float32))
    weight = np.asarray(weight, dtype=np.float32)
    bias = np.asarray(bias, dtype=np.float32)

    # weights: threshold, 1D Winograd G-transform along kw, pack bf16
    w = np.where(np.abs(weight) < SPARSE_TH, np.float32(0.0), weight)
    wt = w.transpose(1, 2, 3, 0)                 # [ci, kh, kw, co]
    G = np.array([[1 / 2, 0, 0],
                  [-1 / 2, -1 / 2, -1 / 2],
                  [-1 / 6, 1 / 6, -1 / 6],
                  [1 / 6, 1 / 3, 2 / 3],
                  [0, 0, 1]], dtype=np.float64)
    u = np.einsum('jl,cklo->jkco', G, wt.astype(np.float64)).astype(np.float32)
    # [pos(5), kh, ci(256), co(256)] -> [ci_in, co_c, pos, kh, ci_c, co_in]
    u6 = u.reshape(5, 3, 2, P, 2, P)             # [pos, kh, ci_c, ci_in, co_c, co_in]
    up = np.ascontiguousarray(
        u6.transpose(3, 4, 0, 1, 2, 5).reshape(P, 60 * P)
    ).astype(ml_dtypes.bfloat16)
    b2 = np.ascontiguousarray(bias.reshape(2, P).T)   # [co_in, co_chunk]

    xb = x.astype(ml_dtypes.bfloat16)
    in_maps = []
    for i in range(N_CORES):
        xc = xb[i * IMGS_PER_CORE:(i + 1) * IMGS_PER_CORE]   # [4,56,56,256]
        # mod-3 de-interleave of width, padded to 19 cols per residue
        xp = np.zeros((IMGS_PER_CORE, H, 3, NW, 2, P), dtype=ml_dtypes.bfloat16)
        for r in range(3):
            nn = len(range(r, W, 3))
            xp[:, :, r, :nn] = xc[:, :, r::3].reshape(IMGS_PER_CORE, H, nn, 2, P)
        # -> [ci_in, img, ci_c, r, h, w19]
        xt_i = np.ascontiguousarray(
            xp.transpose(5, 0, 4, 2, 1, 3).reshape(P, IMGS_PER_CORE * XCOLS_IMG)
        )
        in_maps.append({"xt": xt_i, "up": up, "b2": b2})
    return in_maps


def kernel(x, weight, bias):
    global _NC_CACHE, LAST, _last_in_maps
    in_maps = _marshal(x, weight, bias)

    if _NC_CACHE is None:
        _NC_CACHE = _build_module()
    nc = _NC_CACHE
    _last_in_maps = in_maps

    LAST = run_bass_kernel_spmd(
        nc, in_maps, core_ids=list(range(N_CORES)), trace=TRACE
    )

    out = np.empty((32, OH, OW, CO), np.float32)
    for i in range(N_CORES):
        ytc = np.asarray(LAST.results[i]["yt"]).astype(np.float32)  # [256, 4*2916]
        # cols: [img, blk, i(3), row(27), t(18)]
        y7 = ytc.reshape(2, P, IMGS_PER_CORE, N_BLKS, 3, ROWS_PER_BLK, NT)
        # oh = blk*27+row ; ow = 3t+i ; co = chunk*128+co_in
        out[i * IMGS_PER_CORE:(i + 1) * IMGS_PER_CORE] = (
            y7.transpose(2, 3, 5, 6, 4, 0, 1)
            .reshape(IMGS_PER_CORE, OH, OW, CO)
        )
    return out
